# revision 30
# baseline (speedup 1.0000x reference)
"""Trainium2 Bass kernel: causal self-attention (modded-nanogpt style),
tensor-parallel over heads across 8 NeuronCores with an on-device AllToAll
re-shard before the output projection.

Self-contained: hardcodes B=1, T=4096, D=1024, H=8, Hd=128, scale=0.12.

Per-core program (core = head), processed in 8 groups of four 128-row tiles:
  qkv_stage(g)   4x[ qkv matmuls (xT tile stationary), lambda-mix of v,
                     sum-of-squares stats (ACT Square + accum) ]
  attn_chunk(g-1) S^T = kT.T@qT per 128-key-block; ACT exp out of PSUM
                 (2^-12 folded into the bias keeps fp16 in range and cancels
                 in the normalize); denominator via ones-matmul; normalize =
                 PE row-broadcast + DVE fast reciprocal + one multiply
  norm_stage(g)  batched rsqrt via DVE integer magic + 2 Newton steps (no
                 ACT table loads), q/k normalize, RoPE (batched 4-tile DVE
                 ops), PE transposes
Then two half-AllToAlls re-shard heads -> sequence; the output projection for
the first query half overlaps the second AllToAll.
"""

import os
import sys

sys.path.insert(0, "/opt/trn_rl_repo")

from contextlib import ExitStack

import numpy as np

import concourse.bass as bass
import concourse.bacc as bacc
import concourse.mybir as mybir
import concourse.tile as tile
from concourse.bass_utils import run_bass_kernel_spmd
from concourse.masks import make_identity

N_CORES = 8
T = 4096
D = 1024
H = 8
HD = 128
ATTN_SCALE = 0.12
P = 128
TCH = 512
HALF = TCH // 2
NT = T // P          # 32 t-tiles
NC_CH = T // TCH     # 8 chunks / tile groups
SHARD = T // N_CORES
QUARTER = HD // 4

F32 = mybir.dt.float32
I32 = mybir.dt.int32
_MODE = os.environ.get("KBASS_MM_DT", "f16")
MMD = {"f32r": mybir.dt.float32r, "f16": mybir.dt.float16,
       "f32": F32}[_MODE]
NP_MMD = {"f32r": np.float32, "f16": np.float16, "f32": np.float32}[_MODE]
# exp(s - 12*ln2) = 2^-12 * exp(s): keeps fp16 exp values and their fp16
# partial sums in range; the scaling cancels in the softmax normalize.
EXP_BIAS = -8.317766166719343 if _MODE == "f16" else 0.0
RSQRT_MAGIC = 0x5F3759DF

_cached = {}


def build_module():
    nc = bacc.Bacc("TRN2", target_bir_lowering=False, debug=False,
                   num_devices=N_CORES)

    x_t = nc.dram_tensor("x_t", [D, T], MMD, kind="ExternalInput")
    w_qkv = nc.dram_tensor("w_qkv", [D, 3 * HD], MMD, kind="ExternalInput")
    # host pre-transposed: [P, NT*QUARTER] with (p, n*Q+e) = table[n*P+p, e]
    cos_t = nc.dram_tensor("cos_t", [P, NT * QUARTER], MMD,
                           kind="ExternalInput")
    sin_t = nc.dram_tensor("sin_t", [P, NT * QUARTER], MMD,
                           kind="ExternalInput")
    ve_h = nc.dram_tensor("ve_h", [T, HD], F32, kind="ExternalInput")
    lam = nc.dram_tensor("lam", [P, 2], F32, kind="ExternalInput")
    cpw = nc.dram_tensor("cpw", [D, D], MMD, kind="ExternalInput")
    y_shard = nc.dram_tensor("y_shard", [SHARD, D], F32, kind="ExternalOutput")

    with tile.TileContext(nc) as tc, nc.allow_low_precision(
            reason="reduced-precision matmul operands"), ExitStack() as ctx:
        const = ctx.enter_context(tc.tile_pool(name="const", bufs=1))
        wqkv_pool = ctx.enter_context(tc.tile_pool(name="wqkv", bufs=1))
        big = ctx.enter_context(tc.tile_pool(name="big", bufs=1))
        xt_pool = ctx.enter_context(tc.tile_pool(name="xt", bufs=6))
        xt0_pool = ctx.enter_context(tc.tile_pool(name="xt0", bufs=8))
        ve_pool = ctx.enter_context(tc.tile_pool(name="vein", bufs=3))
        scr_pool = ctx.enter_context(tc.tile_pool(name="scr", bufs=6))
        stat_pool = ctx.enter_context(tc.tile_pool(name="stat", bufs=4))
        qk_pool = ctx.enter_context(tc.tile_pool(name="qksb", bufs=12))
        tin_pool = ctx.enter_context(tc.tile_pool(name="tin", bufs=8))
        exp_pool = ctx.enter_context(tc.tile_pool(name="exp", bufs=5))
        osb_pool = ctx.enter_context(tc.tile_pool(name="osb", bufs=2))
        acc_pool = ctx.enter_context(tc.tile_pool(name="acc", bufs=2))
        rro_pool = ctx.enter_context(tc.tile_pool(name="rro", bufs=6))
        cpw_pool = ctx.enter_context(tc.tile_pool(name="cpw", bufs=1))
        ps = ctx.enter_context(tc.tile_pool(name="ps", bufs=5, space="PSUM"))
        psy = ctx.enter_context(tc.tile_pool(name="psy", bufs=3, space="PSUM"))
        dram = ctx.enter_context(tc.tile_pool(name="dram", bufs=1,
                                              space="DRAM"))

        # ---- weights + tables first so their DMAs lead the queues ----
        # per-k tiles: the first qkv matmul starts after one 96KB slice lands
        wqkv_k = []
        for k in range(D // P):
            wt = wqkv_pool.tile([P, 3 * HD], MMD, name=f"wqkv{k}")
            nc.scalar.dma_start(out=wt[:],
                                in_=w_qkv.ap()[k * P:(k + 1) * P, :])
            wqkv_k.append(wt)
        lam_sb = const.tile([P, 2], F32)
        nc.scalar.dma_start(out=lam_sb[:], in_=lam.ap())
        # rope tables on gpsimd: their slow 64B-line transfers must not sit
        # ahead of the first xt loads on the sync queue
        cos_sb = const.tile([P, NT, QUARTER], MMD)
        nc.gpsimd.dma_start(out=cos_sb[:],
                            in_=cos_t.ap().rearrange("p (n e) -> p n e", n=NT))
        sin_sb = const.tile([P, NT, QUARTER], MMD)
        nc.gpsimd.dma_start(out=sin_sb[:],
                            in_=sin_t.ap().rearrange("p (n e) -> p n e", n=NT))

        # ---- constants ----
        ones_f = const.tile([P, 1], F32)
        nc.vector.memset(ones_f[:], 1.0)
        ones_col = const.tile([P, 1], MMD)
        nc.scalar.copy(ones_col[:], ones_f[:])
        ones_row_f = const.tile([1, P], F32)
        nc.vector.memset(ones_row_f[:], 1.0)
        ones_row = const.tile([1, P], MMD)
        nc.scalar.copy(ones_row[:], ones_row_f[:])
        expb_col = const.tile([P, 1], F32)
        nc.vector.memset(expb_col[:], EXP_BIAS)
        mk_f = const.tile([P, P], F32)
        nc.vector.memset(mk_f[:], 1.0)
        nc.gpsimd.affine_select(
            out=mk_f[:], in_=mk_f[:],
            compare_op=mybir.AluOpType.is_ge, fill=0.0,
            base=0, channel_multiplier=-1, pattern=[[1, P]])
        mask_tri = const.tile([P, P], MMD, name="mask_tri")
        nc.scalar.copy(mask_tri[:], mk_f[:])
        ident_f = const.tile([P, P], F32)
        make_identity(nc, ident_f)
        ident = const.tile([P, P], MMD)
        nc.scalar.copy(ident[:], ident_f[:])

        # ---- persistent per-block tensors (separate tiles => precise deps)
        kT_t = [big.tile([P, P], MMD, name=f"kT{j}") for j in range(NT)]
        v_t = [big.tile([P, HD], MMD, name=f"v{j}") for j in range(NT)]
        qT_c = [big.tile([P, TCH], MMD, name=f"qT{c}") for c in range(NC_CH)]
        yT_c = [big.tile([P, TCH], MMD, name=f"yT{c}") for c in range(NC_CH)]

        # interleaved output ownership: core j owns rows c*512+j*64..+64 of
        # every chunk c, so chunk c's yT can AllToAll right after its
        # normalize — 7 of the 8 collectives hide under attention compute.
        SL = TCH // N_CORES  # 64 query columns per core per chunk
        cc_in_c = [dram.tile([N_CORES * P * SL], MMD, name=f"cc_in{c}")
                   for c in range(NC_CH)]
        cc_out_c = [dram.tile([N_CORES * P * SL], MMD, name=f"cc_out{c}")
                    for c in range(NC_CH)]
        # yall_p[i][e, h, s, f]: head h's columns for shard rows 128i..128i+127
        # (chunks 2i+s) — one tile per outproj tile keeps dependencies exact
        yall_p = [big.tile([P, H, 2, SL], MMD, name=f"yallp{i}")
                  for i in range(4)]

        # full-size warm-up AllToAll: spins up the CC stream + DMA rings
        # early (behind the kernel-entry barrier) so the per-chunk re-shard
        # collectives run at the warm transfer rate.
        cc_wu_in = dram.tile([N_CORES * P * SL], MMD, name="cc_wu_in")
        cc_wu_out = dram.tile([N_CORES * P * SL], MMD, name="cc_wu_out")
        wu_sb = const.tile([P, TCH], MMD)
        nc.vector.memset(wu_sb[:], 0.0)
        nc.gpsimd.dma_start(
            out=cc_wu_in[:].rearrange("(p f) -> p f", p=P), in_=wu_sb[:])
        nc.gpsimd.collective_compute(
            "AllToAll", mybir.AluOpType.bypass,
            replica_groups=[list(range(N_CORES))],
            ins=[cc_wu_in[:].opt()], outs=[cc_wu_out[:].opt()])

        xt_tiles = {}
        xt0_perk = []

        def ensure_xt(i):  # i even: tile pair (i, i+1)
            if i >= NT:
                return
            if i == 0:
                # first pair arrives as per-k slices so the very first qkv
                # matmul starts after one 64KB transfer, not 512KB
                if not xt0_perk:
                    for k in range(D // P):
                        t = xt0_pool.tile([P, 2 * P], MMD, tag="xtk",
                                          name=f"xt0k{k}")
                        nc.sync.dma_start(
                            out=t[:],
                            in_=x_t.ap()[k * P:(k + 1) * P, 0:2 * P])
                        xt0_perk.append(t)
                return
            if i in xt_tiles:
                return
            xt = xt_pool.tile([P, D // P, 2 * P], MMD, tag="xt",
                              name=f"xt{i}")
            nc.sync.dma_start(
                out=xt[:],
                in_=x_t.ap().rearrange("(k p) t -> p k t", p=P)
                    [:, :, i * P:(i + 2) * P])
            xt_tiles[i] = xt

        def xt_slice(i, k):
            xoff = (i % 2) * P
            if i < 2:
                return xt0_perk[k][:, xoff:xoff + P]
            return xt_tiles[i - i % 2][:, k, xoff:xoff + P]

        pending_den = []

        def flush_den():
            # normalize + ship one pending chunk: row copy out of PSUM (ACT),
            # gpsimd partition broadcast, wide DVE reciprocal, one multiply.
            if not pending_den:
                return
            ps_y, ps_r, c = pending_den.pop(0)
            rrow = rro_pool.tile([1, TCH], MMD, tag="rrow", name=f"rrow{c}")
            nc.scalar.copy(rrow[:], ps_r[:])
            # broadcast the denominator row across partitions on the PE
            # (ones ⊗ row): keeps gpsimd out of the tail-critical chain
            ps_bc = ps.tile([P, TCH], F32, tag="ps", name=f"psbc{c}")
            nc.tensor.matmul(ps_bc[:], ones_row[:], rrow[:],
                             start=True, stop=True)
            rc_sb = rro_pool.tile([P, TCH], F32, tag="rc", name=f"rc{c}")
            nc.vector.reciprocal_approx_fast(out=rc_sb[:], in_=ps_bc[:])
            nc.vector.tensor_mul(yT_c[c][:], ps_y[:], rc_sb[:])
            # ship slot-major, AllToAll to the owning cores, gather into yall
            nc.sync.dma_start(
                out=cc_in_c[c][:].rearrange("(l p f) -> p l f",
                                            l=N_CORES, p=P),
                in_=yT_c[c][:].rearrange("p (l f) -> p l f", l=N_CORES))
            nc.gpsimd.collective_compute(
                "AllToAll", mybir.AluOpType.bypass,
                replica_groups=[list(range(N_CORES))],
                ins=[cc_in_c[c][:].opt()], outs=[cc_out_c[c][:].opt()])
            # gather on gpsimd: its wait for the AllToAll must not block the
            # sync queue's xt prefetches
            nc.gpsimd.dma_start(
                out=yall_p[c // 2][:, :, c % 2, :],
                in_=cc_out_c[c][:].rearrange("(j p f) -> p j f",
                                             j=N_CORES, p=P))

        def qkv_stage(g):
            ensure_xt(4 * g)
            ensure_xt(4 * g + 2)
            ve_g = ve_pool.tile([P, 4, HD], F32, tag="ve", name=f"ve{g}")
            nc.gpsimd.dma_start(
                out=ve_g[:],
                in_=ve_h.ap().rearrange("(n p) e -> p n e", p=P)
                    [:, 4 * g:4 * g + 4, :])
            # ssq_g[:, 2i:2i+2] = [sum q^2, sum k^2] for tile 4g+i
            ssq_g = stat_pool.tile([P, 8], F32, tag="ssq", name=f"ssq{g}")
            ps_qkvs = []
            for ii in range(4):
                i = 4 * g + ii
                ensure_xt(i - i % 2)
                ps_qkv = ps.tile([P, 3 * HD], F32, tag="ps",
                                 name=f"psqkv{i}")
                for k in range(D // P):
                    nc.tensor.matmul(ps_qkv[:], xt_slice(i, k),
                                     wqkv_k[k][:],
                                     start=(k == 0), stop=(k == D // P - 1))
                nc.vector.scalar_tensor_tensor(
                    out=v_t[i][:], in0=ps_qkv[:, 2 * HD:3 * HD],
                    scalar=lam_sb[:, 0:1], in1=ve_g[:, ii, :],
                    op0=mybir.AluOpType.mult, op1=mybir.AluOpType.add)
                qk_sb = qk_pool.tile([P, 2 * HD], F32, tag="qksb",
                                     name=f"qksb{i}")
                if ii % 2:
                    nc.vector.tensor_copy(qk_sb[:], ps_qkv[:, 0:2 * HD])
                else:
                    nc.scalar.copy(qk_sb[:], ps_qkv[:, 0:2 * HD])
                sq = scr_pool.tile([P, HD], F32, tag="sq")
                nc.scalar.activation(sq[:], qk_sb[:, 0:HD],
                                     mybir.ActivationFunctionType.Square,
                                     accum_out=ssq_g[:, 2 * ii:2 * ii + 1])
                nc.scalar.activation(sq[:], qk_sb[:, HD:2 * HD],
                                     mybir.ActivationFunctionType.Square,
                                     accum_out=ssq_g[:, 2 * ii + 1:2 * ii + 2])
                ps_qkvs.append(qk_sb)
            # prefetch next group's x tiles
            ensure_xt(4 * g + 4)
            ensure_xt(4 * g + 6)
            return ssq_g, ps_qkvs, ve_g

        def norm_stage(g, ssq_g, ps_qkvs):
            # rsq = 1/sqrt(ssq) batched for the group: integer magic + 2
            # Newton iterations, all on DVE (no ACT table involvement).
            # 1/sqrt(mean) = rsq * sqrt(HD) is folded into the final scales.
            h_i = stat_pool.tile([P, 8], I32, tag="h_i")
            nc.vector.tensor_scalar(
                out=h_i[:], in0=ssq_g[:].bitcast(I32), scalar1=1,
                scalar2=None,
                op0=mybir.AluOpType.logical_shift_right)
            y0 = stat_pool.tile([P, 8], F32, tag="y0")
            nc.vector.tensor_scalar(
                out=y0[:].bitcast(I32), in0=h_i[:], scalar1=-1,
                scalar2=RSQRT_MAGIC,
                op0=mybir.AluOpType.mult, op1=mybir.AluOpType.add)
            t1 = stat_pool.tile([P, 8], F32, tag="t1")
            rsq = stat_pool.tile([P, 8], F32, tag="rsq", name=f"rsq{g}")
            cur = y0
            for it, nxt in ((0, t1), (1, rsq)):
                tt = stat_pool.tile([P, 8], F32, tag=f"tt{it}")
                nc.vector.tensor_mul(tt[:], cur[:], cur[:])
                nc.vector.tensor_mul(tt[:], tt[:], ssq_g[:])
                nc.vector.tensor_scalar(
                    out=tt[:], in0=tt[:], scalar1=-0.5, scalar2=1.5,
                    op0=mybir.AluOpType.mult, op1=mybir.AluOpType.add)
                nc.vector.tensor_mul(nxt[:], cur[:], tt[:])
                cur = nxt

            sq128 = float(np.sqrt(HD))
            for ii in range(4):
                i = 4 * g + ii
                qk_sb = ps_qkvs[ii]
                qkn = tin_pool.tile([P, 2 * HD], MMD, tag="qkn",
                                    name=f"qkn{i}")
                nc.vector.tensor_scalar(
                    out=qkn[:, 0:HD], in0=qk_sb[:, 0:HD],
                    scalar1=rsq[:, 2 * ii:2 * ii + 1],
                    scalar2=ATTN_SCALE * sq128,
                    op0=mybir.AluOpType.mult, op1=mybir.AluOpType.mult)
                nc.vector.tensor_scalar(
                    out=qkn[:, HD:2 * HD], in0=qk_sb[:, HD:2 * HD],
                    scalar1=rsq[:, 2 * ii + 1:2 * ii + 2], scalar2=sq128,
                    op0=mybir.AluOpType.mult, op1=mybir.AluOpType.mult)

                # rope on first-quarter pairs of q AND k in one op each
                def two_rng(tl, col0):
                    src = tl[:]
                    return bass.AP(src.tensor, src.offset + col0,
                                   [list(src.ap[0]), [HD, 2], [1, QUARTER]])

                def cs_b(ap2d):
                    return bass.AP(ap2d.tensor, ap2d.offset,
                                   [list(ap2d.ap[0]), [0, 2],
                                    list(ap2d.ap[-1])])

                x1 = two_rng(qkn, 0)
                x2 = two_rng(qkn, 2 * QUARTER)
                cb_ = cs_b(cos_sb[:, 4 * g + ii, :])
                sb_ = cs_b(sin_sb[:, 4 * g + ii, :])
                a = scr_pool.tile([P, 2, QUARTER], MMD, tag="ropeA")
                b = scr_pool.tile([P, 2, QUARTER], MMD, tag="ropeB")
                c2 = scr_pool.tile([P, 2, QUARTER], MMD, tag="ropeC")
                d2 = scr_pool.tile([P, 2, QUARTER], MMD, tag="ropeD")
                nc.vector.tensor_mul(a[:], x1, cb_)
                nc.vector.tensor_mul(b[:], x2, sb_)
                nc.vector.tensor_mul(c2[:], x2, cb_)
                nc.vector.tensor_mul(d2[:], x1, sb_)
                nc.vector.tensor_add(x1, a[:], b[:])
                nc.vector.tensor_sub(x2, c2[:], d2[:])
                # transpose q,k into [e, t] layout (PE transpose, evict)
                sub = ii * P
                for ei, (src_ap, dst, c0) in enumerate(
                        ((qkn[:, 0:HD], qT_c[g], sub),
                         (qkn[:, HD:2 * HD], kT_t[i], 0))):
                    ps_tr = ps.tile([P, P], MMD, tag="ps")
                    nc.tensor.transpose(ps_tr[:], src_ap, ident[:])
                    if ei:
                        nc.vector.tensor_copy(dst[:, c0:c0 + P], ps_tr[:])
                    else:
                        nc.scalar.copy(dst[:, c0:c0 + P], ps_tr[:])

        def attn_chunk(c):
            # diagonal blocks j=4c+m only touch live query columns >= 128*m;
            # the S matmul / exp / accumulate / AV all restrict to that
            # subrange and a single [P,P] triangular mask covers the edge.
            jmax = 4 * c + 4
            ps_y = psy.tile([P, TCH], F32, tag="psy", name=f"psy{c}")
            acc = acc_pool.tile([P, TCH], MMD, name=f"acc{c}")
            s_psums = {}

            def col0_of(j):
                return max(0, (j - 4 * c)) * P

            def s_mm(j):
                c0 = col0_of(j)
                p_s = ps.tile([P, TCH], F32, tag="ps")
                nc.tensor.matmul(p_s[:, c0:], kT_t[j][:], qT_c[c][:, c0:],
                                 start=True, stop=True)
                return p_s

            s_psums[0] = s_mm(0)
            for j in range(jmax):
                if j + 1 < jmax:
                    s_psums[j + 1] = s_mm(j + 1)
                c0 = col0_of(j)
                p_s = s_psums.pop(j)
                e_sb = exp_pool.tile([P, TCH], MMD)
                nc.scalar.activation(e_sb[:, c0:], p_s[:, c0:],
                                     mybir.ActivationFunctionType.Exp,
                                     bias=expb_col[:])
                if j == 0:
                    # previous chunk's normalize goes behind our first exp so
                    # the ACT queue never stalls on our denominator.
                    flush_den()
                if j >= 4 * c:
                    nc.vector.tensor_mul(e_sb[:, c0:c0 + P],
                                         e_sb[:, c0:c0 + P], mask_tri[:])
                if j == 0:
                    nc.vector.tensor_copy(acc[:], e_sb[:])
                else:
                    nc.vector.tensor_add(acc[:, c0:], acc[:, c0:],
                                         e_sb[:, c0:])
                nc.tensor.matmul(ps_y[:, c0:], v_t[j][:], e_sb[:, c0:],
                                 start=(j == 0), stop=(j == jmax - 1),
                                 skip_group_check=True)
            ps_r = psy.tile([1, TCH], F32, tag="psy", name=f"psr{c}")
            nc.tensor.matmul(ps_r[:], ones_col[:], acc[:],
                             start=True, stop=True)
            pending_den.append((ps_y, ps_r, c))

        cpw_sb = cpw_pool.tile([P, H, D], MMD)

        def outproj_tile(i):
            # rows 128*i..128*(i+1) of this core's shard = chunks 2i, 2i+1
            o_sb = osb_pool.tile([P, D], F32, tag="osb", name=f"osb{i}")
            for dh in range(D // TCH):
                ps_o = ps.tile([P, TCH], F32, tag="ps")
                for h in range(H):
                    nc.tensor.matmul(
                        ps_o[:], yall_p[i][:, h, :, :],
                        cpw_sb[:, h, dh * TCH:(dh + 1) * TCH],
                        start=(h == 0), stop=(h == H - 1))
                osl = o_sb[:, dh * TCH:(dh + 1) * TCH]
                if dh:
                    nc.vector.tensor_copy(osl, ps_o[:])
                else:
                    nc.scalar.copy(osl, ps_o[:])
                nc.sync.dma_start(
                    out=y_shard.ap()[i * P:(i + 1) * P,
                                     dh * TCH:(dh + 1) * TCH],
                    in_=osl)

        # ---- main loop: attn(g-1) | qkv(g+2) | norm(g) dovetail ----
        handles = {0: qkv_stage(0), 1: qkv_stage(1)}
        for g in range(NC_CH):
            if g >= 1:
                attn_chunk(g - 1)
            if g + 2 < NC_CH:
                handles[g + 2] = qkv_stage(g + 2)
            ssq_g, ps_qkvs, ve_g = handles.pop(g)
            norm_stage(g, ssq_g, ps_qkvs)
            if g == 2:  # prefetch output-projection weights mid-flight
                nc.gpsimd.dma_start(
                    out=cpw_sb[:],
                    in_=cpw.ap().rearrange("(h p) d -> p h d", p=P))
            if g >= 6:  # chunks 2i,2i+1 resharded long ago by now
                outproj_tile(g - 6)
        attn_chunk(NC_CH - 1)
        flush_den()
        # tile 2's matmuls cover the last chunk's AllToAll latency; tile 3
        # (chunks 6,7) then finds its data already landed.
        outproj_tile(2)
        outproj_tile(3)

    nc.compile()
    return nc


def _host_prep(x, ve, qkv_w, lambdas, c_proj_w):
    x = np.asarray(x, dtype=np.float32)
    ve = np.asarray(ve, dtype=np.float32)
    qkv_w = np.asarray(qkv_w, dtype=np.float32)
    lambdas = np.asarray(lambdas, dtype=np.float32)
    c_proj_w = np.asarray(c_proj_w, dtype=np.float32)

    xT = np.ascontiguousarray(x[0].T.astype(NP_MMD))
    cpwT = np.ascontiguousarray(c_proj_w.T.astype(NP_MMD))
    lam_b = np.ascontiguousarray(np.broadcast_to(lambdas, (P, 2)))

    angular = (np.float32(1.0 / 1024.0)
               ** np.linspace(0.0, 1.0, QUARTER, dtype=np.float32))
    t = np.arange(T, dtype=np.float32)
    theta = t[:, None] * angular[None, :]
    # [T, Q] -> [P, NT*Q] with (p, n*Q+e) = table[n*P+p, e]
    cos32 = np.ascontiguousarray(
        np.cos(theta).astype(NP_MMD).reshape(NT, P, QUARTER)
        .transpose(1, 0, 2).reshape(P, NT * QUARTER))
    sin32 = np.ascontiguousarray(
        np.sin(theta).astype(NP_MMD).reshape(NT, P, QUARTER)
        .transpose(1, 0, 2).reshape(P, NT * QUARTER))

    in_maps = []
    for h in range(N_CORES):
        sl = slice(h * HD, (h + 1) * HD)
        w_qkvT = np.ascontiguousarray(np.concatenate(
            [qkv_w[0, sl, :].T, qkv_w[1, sl, :].T, qkv_w[2, sl, :].T],
            axis=1).astype(NP_MMD))
        in_maps.append({
            "x_t": xT,
            "w_qkv": w_qkvT,
            "cos_t": cos32,
            "sin_t": sin32,
            "ve_h": np.ascontiguousarray(ve[0][:, sl] * lambdas[1]),
            "lam": lam_b,
            "cpw": cpwT,
        })
    return in_maps


def kernel(x, ve, qkv_w, lambdas, c_proj_w, _trace=False, _trace_kwargs=None):
    if "nc" not in _cached:
        _cached["nc"] = build_module()
    nc = _cached["nc"]
    in_maps = _host_prep(x, ve, qkv_w, lambdas, c_proj_w)
    kw = {}
    if _trace:
        kw = dict(trace=True, **(_trace_kwargs or {}))
    res = run_bass_kernel_spmd(nc, in_maps, core_ids=list(range(N_CORES)),
                               **kw)
    _cached["last_result"] = res
    # core j's shard rows are 64-row slices: shard row 64*c+k holds global
    # row c*512 + j*64 + k (interleaved ownership, see build_module)
    parts = np.stack([res.results[c]["y_shard"] for c in range(N_CORES)])
    out = (parts.reshape(N_CORES, NC_CH, TCH // N_CORES, D)
           .transpose(1, 0, 2, 3).reshape(T, D))
    return out[None].astype(np.float32)


# revision 35
# speedup vs baseline: 1.0540x; 1.0540x over previous
"""Trainium2 Bass kernel: causal self-attention (modded-nanogpt style),
tensor-parallel over heads across 8 NeuronCores with an on-device AllToAll
re-shard before the output projection.

Self-contained: hardcodes B=1, T=4096, D=1024, H=8, Hd=128, scale=0.12.

Per-core program (core = head), processed in 8 groups of four 128-row tiles:
  qkv_stage(g)   4x[ qkv matmuls (xT tile stationary), lambda-mix of v,
                     sum-of-squares stats (ACT Square + accum) ]
  attn_chunk(g-1) S^T = kT.T@qT per 128-key-block; ACT exp out of PSUM
                 (2^-12 folded into the bias keeps fp16 in range and cancels
                 in the normalize); denominator via ones-matmul; normalize =
                 PE row-broadcast + DVE fast reciprocal + one multiply
  norm_stage(g)  batched rsqrt via DVE integer magic + 2 Newton steps (no
                 ACT table loads), q/k normalize, RoPE (batched 4-tile DVE
                 ops), PE transposes
Then two half-AllToAlls re-shard heads -> sequence; the output projection for
the first query half overlaps the second AllToAll.
"""

import os
import sys

sys.path.insert(0, "/opt/trn_rl_repo")

from contextlib import ExitStack

import numpy as np

import concourse.bass as bass
import concourse.bacc as bacc
import concourse.mybir as mybir
import concourse.tile as tile
from concourse.bass_utils import run_bass_kernel_spmd
from concourse.masks import make_identity

N_CORES = 8
T = 4096
D = 1024
H = 8
HD = 128
ATTN_SCALE = 0.12
P = 128
TCH = 512
HALF = TCH // 2
NT = T // P          # 32 t-tiles
NC_CH = T // TCH     # 8 chunks / tile groups
SHARD = T // N_CORES
QUARTER = HD // 4

F32 = mybir.dt.float32
I32 = mybir.dt.int32
_MODE = os.environ.get("KBASS_MM_DT", "f16")
MMD = {"f32r": mybir.dt.float32r, "f16": mybir.dt.float16,
       "f32": F32}[_MODE]
NP_MMD = {"f32r": np.float32, "f16": np.float16, "f32": np.float32}[_MODE]
# exp(s - 12*ln2) = 2^-12 * exp(s): keeps fp16 exp values and their fp16
# partial sums in range; the scaling cancels in the softmax normalize.
EXP_BIAS = -8.317766166719343 if _MODE == "f16" else 0.0
RSQRT_MAGIC = 0x5F3759DF

_cached = {}


def build_module():
    nc = bacc.Bacc("TRN2", target_bir_lowering=False, debug=False,
                   num_devices=N_CORES)

    x_t = nc.dram_tensor("x_t", [D, T], MMD, kind="ExternalInput")
    w_qkv = nc.dram_tensor("w_qkv", [D, 3 * HD], MMD, kind="ExternalInput")
    # host pre-transposed: [P, NT*QUARTER] with (p, n*Q+e) = table[n*P+p, e]
    cos_t = nc.dram_tensor("cos_t", [P, NT * QUARTER], MMD,
                           kind="ExternalInput")
    sin_t = nc.dram_tensor("sin_t", [P, NT * QUARTER], MMD,
                           kind="ExternalInput")
    ve_h = nc.dram_tensor("ve_h", [T, HD], F32, kind="ExternalInput")
    lam = nc.dram_tensor("lam", [P, 2], F32, kind="ExternalInput")
    cpw = nc.dram_tensor("cpw", [D, D], MMD, kind="ExternalInput")
    y_shard = nc.dram_tensor("y_shard", [SHARD, D], F32, kind="ExternalOutput")

    with tile.TileContext(nc) as tc, nc.allow_low_precision(
            reason="reduced-precision matmul operands"), ExitStack() as ctx:
        const = ctx.enter_context(tc.tile_pool(name="const", bufs=1))
        wqkv_pool = ctx.enter_context(tc.tile_pool(name="wqkv", bufs=1))
        big = ctx.enter_context(tc.tile_pool(name="big", bufs=1))
        xt_pool = ctx.enter_context(tc.tile_pool(name="xt", bufs=6))
        xt0_pool = ctx.enter_context(tc.tile_pool(name="xt0", bufs=8))
        ve_pool = ctx.enter_context(tc.tile_pool(name="vein", bufs=3))
        scr_pool = ctx.enter_context(tc.tile_pool(name="scr", bufs=6))
        stat_pool = ctx.enter_context(tc.tile_pool(name="stat", bufs=4))
        qk_pool = ctx.enter_context(tc.tile_pool(name="qksb", bufs=12))
        tin_pool = ctx.enter_context(tc.tile_pool(name="tin", bufs=8))
        exp_pool = ctx.enter_context(tc.tile_pool(name="exp", bufs=5))
        osb_pool = ctx.enter_context(tc.tile_pool(name="osb", bufs=2))
        acc_pool = ctx.enter_context(tc.tile_pool(name="acc", bufs=2))
        rro_pool = ctx.enter_context(tc.tile_pool(name="rro", bufs=6))
        cpw_pool = ctx.enter_context(tc.tile_pool(name="cpw", bufs=1))
        ps = ctx.enter_context(tc.tile_pool(name="ps", bufs=5, space="PSUM"))
        psy = ctx.enter_context(tc.tile_pool(name="psy", bufs=3, space="PSUM"))
        dram = ctx.enter_context(tc.tile_pool(name="dram", bufs=1,
                                              space="DRAM"))

        # ---- weights + tables first so their DMAs lead the queues ----
        # per-k tiles: the first qkv matmul starts after one 96KB slice lands
        wqkv_k = []
        for k in range(D // P):
            wt = wqkv_pool.tile([P, 3 * HD], MMD, name=f"wqkv{k}")
            nc.scalar.dma_start(out=wt[:],
                                in_=w_qkv.ap()[k * P:(k + 1) * P, :])
            wqkv_k.append(wt)
        lam_sb = const.tile([P, 2], F32)
        nc.scalar.dma_start(out=lam_sb[:], in_=lam.ap())
        # rope tables on gpsimd: their slow 64B-line transfers must not sit
        # ahead of the first xt loads on the sync queue
        cos_sb = const.tile([P, NT, QUARTER], MMD)
        nc.gpsimd.dma_start(out=cos_sb[:],
                            in_=cos_t.ap().rearrange("p (n e) -> p n e", n=NT))
        sin_sb = const.tile([P, NT, QUARTER], MMD)
        nc.gpsimd.dma_start(out=sin_sb[:],
                            in_=sin_t.ap().rearrange("p (n e) -> p n e", n=NT))

        # ---- constants ----
        ones_f = const.tile([P, 1], F32)
        nc.vector.memset(ones_f[:], 1.0)
        ones_col = const.tile([P, 1], MMD)
        nc.scalar.copy(ones_col[:], ones_f[:])
        ones_row_f = const.tile([1, P], F32)
        nc.vector.memset(ones_row_f[:], 1.0)
        ones_row = const.tile([1, P], MMD)
        nc.scalar.copy(ones_row[:], ones_row_f[:])
        expb_col = const.tile([P, 1], F32)
        nc.vector.memset(expb_col[:], EXP_BIAS)
        mk_f = const.tile([P, P], F32)
        nc.vector.memset(mk_f[:], 1.0)
        nc.gpsimd.affine_select(
            out=mk_f[:], in_=mk_f[:],
            compare_op=mybir.AluOpType.is_ge, fill=0.0,
            base=0, channel_multiplier=-1, pattern=[[1, P]])
        mask_tri = const.tile([P, P], MMD, name="mask_tri")
        nc.scalar.copy(mask_tri[:], mk_f[:])
        ident_f = const.tile([P, P], F32)
        make_identity(nc, ident_f)
        ident = const.tile([P, P], MMD)
        nc.scalar.copy(ident[:], ident_f[:])

        # ---- persistent per-block tensors (separate tiles => precise deps)
        kT_t = [big.tile([P, P], MMD, name=f"kT{j}") for j in range(NT)]
        v_t = [big.tile([P, HD], MMD, name=f"v{j}") for j in range(NT)]
        qT_c = [big.tile([P, TCH], MMD, name=f"qT{c}") for c in range(NC_CH)]
        yT_c = [big.tile([P, TCH], MMD, name=f"yT{c}") for c in range(NC_CH)]

        # interleaved output ownership: core j owns rows c*512+j*64..+64 of
        # every chunk c, so chunk c's yT can AllToAll right after its
        # normalize — 7 of the 8 collectives hide under attention compute.
        SL = TCH // N_CORES  # 64 query columns per core per chunk
        cc_in_c = [dram.tile([N_CORES * P * SL], MMD, name=f"cc_in{c}")
                   for c in range(NC_CH)]
        cc_out_c = [dram.tile([N_CORES * P * SL], MMD, name=f"cc_out{c}")
                    for c in range(NC_CH)]
        # yall_p[i][e, h, s, f]: head h's columns for shard rows 128i..128i+127
        # (chunks 2i+s) — one tile per outproj tile keeps dependencies exact
        yall_p = [big.tile([P, H, 2, SL], MMD, name=f"yallp{i}")
                  for i in range(4)]

        # full-size warm-up AllToAll: spins up the CC stream + DMA rings
        # early (behind the kernel-entry barrier) so the per-chunk re-shard
        # collectives run at the warm transfer rate.
        cc_wu_in = dram.tile([N_CORES * P * SL], MMD, name="cc_wu_in")
        cc_wu_out = dram.tile([N_CORES * P * SL], MMD, name="cc_wu_out")
        wu_sb = const.tile([P, TCH], MMD)
        nc.vector.memset(wu_sb[:], 0.0)
        nc.gpsimd.dma_start(
            out=cc_wu_in[:].rearrange("(p f) -> p f", p=P), in_=wu_sb[:])
        nc.gpsimd.collective_compute(
            "AllToAll", mybir.AluOpType.bypass,
            replica_groups=[list(range(N_CORES))],
            ins=[cc_wu_in[:].opt()], outs=[cc_wu_out[:].opt()])

        xt_tiles = {}
        xt0_perk = []

        def ensure_xt(i):  # i even: tile pair (i, i+1)
            if i >= NT:
                return
            if i == 0:
                # first pair arrives as per-k slices so the very first qkv
                # matmul starts after one 64KB transfer, not 512KB
                if not xt0_perk:
                    for k in range(D // P):
                        t = xt0_pool.tile([P, 2 * P], MMD, tag="xtk",
                                          name=f"xt0k{k}")
                        nc.sync.dma_start(
                            out=t[:],
                            in_=x_t.ap()[k * P:(k + 1) * P, 0:2 * P])
                        xt0_perk.append(t)
                return
            if i in xt_tiles:
                return
            xt = xt_pool.tile([P, D // P, 2 * P], MMD, tag="xt",
                              name=f"xt{i}")
            nc.sync.dma_start(
                out=xt[:],
                in_=x_t.ap().rearrange("(k p) t -> p k t", p=P)
                    [:, :, i * P:(i + 2) * P])
            xt_tiles[i] = xt

        def xt_slice(i, k):
            xoff = (i % 2) * P
            if i < 2:
                return xt0_perk[k][:, xoff:xoff + P]
            return xt_tiles[i - i % 2][:, k, xoff:xoff + P]

        pending_den = []

        def flush_den():
            # normalize + ship one pending chunk: row copy out of PSUM (ACT),
            # gpsimd partition broadcast, wide DVE reciprocal, one multiply.
            if not pending_den:
                return
            ps_y, ps_r, c = pending_den.pop(0)
            rrow = rro_pool.tile([1, TCH], MMD, tag="rrow", name=f"rrow{c}")
            nc.scalar.copy(rrow[:], ps_r[:])
            # broadcast the denominator row across partitions on the PE
            # (ones ⊗ row): keeps gpsimd out of the tail-critical chain
            ps_bc = ps.tile([P, TCH], F32, tag="ps", name=f"psbc{c}")
            nc.tensor.matmul(ps_bc[:], ones_row[:], rrow[:],
                             start=True, stop=True)
            rc_sb = rro_pool.tile([P, TCH], F32, tag="rc", name=f"rc{c}")
            nc.vector.reciprocal_approx_fast(out=rc_sb[:], in_=ps_bc[:])
            nc.vector.tensor_mul(yT_c[c][:], ps_y[:], rc_sb[:])
            # ship slot-major, AllToAll to the owning cores, gather into yall
            nc.sync.dma_start(
                out=cc_in_c[c][:].rearrange("(l p f) -> p l f",
                                            l=N_CORES, p=P),
                in_=yT_c[c][:].rearrange("p (l f) -> p l f", l=N_CORES))
            nc.gpsimd.collective_compute(
                "AllToAll", mybir.AluOpType.bypass,
                replica_groups=[list(range(N_CORES))],
                ins=[cc_in_c[c][:].opt()], outs=[cc_out_c[c][:].opt()])
            # gather on gpsimd: its wait for the AllToAll must not block the
            # sync queue's xt prefetches
            nc.gpsimd.dma_start(
                out=yall_p[c // 2][:, :, c % 2, :],
                in_=cc_out_c[c][:].rearrange("(j p f) -> p j f",
                                             j=N_CORES, p=P))

        def qkv_pre(g):
            # DMAs lead: issued before the attention chunk they feed under
            ensure_xt(4 * g)
            ensure_xt(4 * g + 2)
            ve_g = ve_pool.tile([P, 4, HD], F32, tag="ve", name=f"ve{g}")
            nc.gpsimd.dma_start(
                out=ve_g[:],
                in_=ve_h.ap().rearrange("(n p) e -> p n e", p=P)
                    [:, 4 * g:4 * g + 4, :])
            # ssq[:, 2i:2i+2] = [sum q^2, sum k^2] for tile 4g+i
            ssq_g = stat_pool.tile([P, 8], F32, tag="ssq", name=f"ssq{g}")
            return {"ve": ve_g, "ssq": ssq_g, "qk": []}

        def qkv_tile(g, st, ii):
            i = 4 * g + ii
            ps_qkv = ps.tile([P, 3 * HD], F32, tag="ps", name=f"psqkv{i}")
            for k in range(D // P):
                nc.tensor.matmul(ps_qkv[:], xt_slice(i, k), wqkv_k[k][:],
                                 start=(k == 0), stop=(k == D // P - 1))
            nc.vector.scalar_tensor_tensor(
                out=v_t[i][:], in0=ps_qkv[:, 2 * HD:3 * HD],
                scalar=lam_sb[:, 0:1], in1=st["ve"][:, ii, :],
                op0=mybir.AluOpType.mult, op1=mybir.AluOpType.add)
            qk_sb = qk_pool.tile([P, 2 * HD], F32, tag="qksb",
                                 name=f"qksb{i}")
            if ii % 2:
                nc.vector.tensor_copy(qk_sb[:], ps_qkv[:, 0:2 * HD])
            else:
                nc.scalar.copy(qk_sb[:], ps_qkv[:, 0:2 * HD])
            sq = scr_pool.tile([P, HD], F32, tag="sq")
            nc.scalar.activation(sq[:], qk_sb[:, 0:HD],
                                 mybir.ActivationFunctionType.Square,
                                 accum_out=st["ssq"][:, 2 * ii:2 * ii + 1])
            nc.scalar.activation(sq[:], qk_sb[:, HD:2 * HD],
                                 mybir.ActivationFunctionType.Square,
                                 accum_out=st["ssq"][:, 2 * ii + 1:2 * ii + 2])
            st["qk"].append(qk_sb)
            if ii == 3:  # prefetch next group's x tiles
                ensure_xt(4 * g + 4)
                ensure_xt(4 * g + 6)

        def norm_pre(g, st):
            # rsq = 1/sqrt(ssq) batched for the group: integer magic + 2
            # Newton iterations, all on DVE (no ACT table involvement).
            # 1/sqrt(mean) = rsq * sqrt(HD) is folded into the final scales.
            ssq_g = st["ssq"]
            h_i = stat_pool.tile([P, 8], I32, tag="h_i")
            nc.vector.tensor_scalar(
                out=h_i[:], in0=ssq_g[:].bitcast(I32), scalar1=1,
                scalar2=None,
                op0=mybir.AluOpType.logical_shift_right)
            y0 = stat_pool.tile([P, 8], F32, tag="y0")
            nc.vector.tensor_scalar(
                out=y0[:].bitcast(I32), in0=h_i[:], scalar1=-1,
                scalar2=RSQRT_MAGIC,
                op0=mybir.AluOpType.mult, op1=mybir.AluOpType.add)
            t1 = stat_pool.tile([P, 8], F32, tag="t1")
            rsq = stat_pool.tile([P, 8], F32, tag="rsq", name=f"rsq{g}")
            cur = y0
            for it, nxt in ((0, t1), (1, rsq)):
                tt = stat_pool.tile([P, 8], F32, tag=f"tt{it}")
                nc.vector.tensor_mul(tt[:], cur[:], cur[:])
                nc.vector.tensor_mul(tt[:], tt[:], ssq_g[:])
                nc.vector.tensor_scalar(
                    out=tt[:], in0=tt[:], scalar1=-0.5, scalar2=1.5,
                    op0=mybir.AluOpType.mult, op1=mybir.AluOpType.add)
                nc.vector.tensor_mul(nxt[:], cur[:], tt[:])
                cur = nxt
            st["rsq"] = rsq

        def norm_tile(g, st, ii):
            sq128 = float(np.sqrt(HD))
            rsq = st["rsq"]
            if True:
                i = 4 * g + ii
                qk_sb = st["qk"][ii]
                qkn = tin_pool.tile([P, 2 * HD], MMD, tag="qkn",
                                    name=f"qkn{i}")
                nc.vector.tensor_scalar(
                    out=qkn[:, 0:HD], in0=qk_sb[:, 0:HD],
                    scalar1=rsq[:, 2 * ii:2 * ii + 1],
                    scalar2=ATTN_SCALE * sq128,
                    op0=mybir.AluOpType.mult, op1=mybir.AluOpType.mult)
                nc.vector.tensor_scalar(
                    out=qkn[:, HD:2 * HD], in0=qk_sb[:, HD:2 * HD],
                    scalar1=rsq[:, 2 * ii + 1:2 * ii + 2], scalar2=sq128,
                    op0=mybir.AluOpType.mult, op1=mybir.AluOpType.mult)

                # rope on first-quarter pairs of q AND k in one op each
                def two_rng(tl, col0):
                    src = tl[:]
                    return bass.AP(src.tensor, src.offset + col0,
                                   [list(src.ap[0]), [HD, 2], [1, QUARTER]])

                def cs_b(ap2d):
                    return bass.AP(ap2d.tensor, ap2d.offset,
                                   [list(ap2d.ap[0]), [0, 2],
                                    list(ap2d.ap[-1])])

                x1 = two_rng(qkn, 0)
                x2 = two_rng(qkn, 2 * QUARTER)
                cb_ = cs_b(cos_sb[:, 4 * g + ii, :])
                sb_ = cs_b(sin_sb[:, 4 * g + ii, :])
                a = scr_pool.tile([P, 2, QUARTER], MMD, tag="ropeA")
                b = scr_pool.tile([P, 2, QUARTER], MMD, tag="ropeB")
                c2 = scr_pool.tile([P, 2, QUARTER], MMD, tag="ropeC")
                d2 = scr_pool.tile([P, 2, QUARTER], MMD, tag="ropeD")
                nc.vector.tensor_mul(a[:], x1, cb_)
                nc.vector.tensor_mul(b[:], x2, sb_)
                nc.vector.tensor_mul(c2[:], x2, cb_)
                nc.vector.tensor_mul(d2[:], x1, sb_)
                nc.vector.tensor_add(x1, a[:], b[:])
                nc.vector.tensor_sub(x2, c2[:], d2[:])
                # transpose q,k into [e, t] layout (PE transpose, evict)
                sub = ii * P
                for ei, (src_ap, dst, c0) in enumerate(
                        ((qkn[:, 0:HD], qT_c[g], sub),
                         (qkn[:, HD:2 * HD], kT_t[i], 0))):
                    ps_tr = ps.tile([P, P], MMD, tag="ps")
                    nc.tensor.transpose(ps_tr[:], src_ap, ident[:])
                    if ei:
                        nc.vector.tensor_copy(dst[:, c0:c0 + P], ps_tr[:])
                    else:
                        nc.scalar.copy(dst[:, c0:c0 + P], ps_tr[:])

        def attn_chunk(c, feed=()):
            # diagonal blocks j=4c+m only touch live query columns >= 128*m;
            # the S matmul / exp / accumulate / AV all restrict to that
            # subrange and a single [P,P] triangular mask covers the edge.
            feed = list(feed)
            jmax = 4 * c + 4
            ps_y = psy.tile([P, TCH], F32, tag="psy", name=f"psy{c}")
            acc = acc_pool.tile([P, TCH], MMD, name=f"acc{c}")
            s_psums = {}

            def col0_of(j):
                return max(0, (j - 4 * c)) * P

            def s_mm(j):
                c0 = col0_of(j)
                p_s = ps.tile([P, TCH], F32, tag="ps")
                nc.tensor.matmul(p_s[:, c0:], kT_t[j][:], qT_c[c][:, c0:],
                                 start=True, stop=True)
                return p_s

            per = -(-len(feed) // jmax) if feed else 0
            s_psums[0] = s_mm(0)
            for j in range(jmax):
                if j + 1 < jmax:
                    s_psums[j + 1] = s_mm(j + 1)
                c0 = col0_of(j)
                p_s = s_psums.pop(j)
                e_sb = exp_pool.tile([P, TCH], MMD)
                nc.scalar.activation(e_sb[:, c0:], p_s[:, c0:],
                                     mybir.ActivationFunctionType.Exp,
                                     bias=expb_col[:])
                if j == 0:
                    # previous chunk's normalize goes behind our first exp so
                    # the ACT queue never stalls on our denominator.
                    flush_den()
                if j >= 4 * c:
                    nc.vector.tensor_mul(e_sb[:, c0:c0 + P],
                                         e_sb[:, c0:c0 + P], mask_tri[:])
                if j == 0:
                    nc.vector.tensor_copy(acc[:], e_sb[:])
                else:
                    nc.vector.tensor_add(acc[:, c0:], acc[:, c0:],
                                         e_sb[:, c0:])
                nc.tensor.matmul(ps_y[:, c0:], v_t[j][:], e_sb[:, c0:],
                                 start=(j == 0), stop=(j == jmax - 1),
                                 skip_group_check=True)
                # interleave co-scheduled work units between blocks so their
                # PSUM evictions never queue behind a whole chunk of exps
                for _ in range(per):
                    if feed:
                        feed.pop(0)()
            while feed:
                feed.pop(0)()
            ps_r = psy.tile([1, TCH], F32, tag="psy", name=f"psr{c}")
            nc.tensor.matmul(ps_r[:], ones_col[:], acc[:],
                             start=True, stop=True)
            pending_den.append((ps_y, ps_r, c))

        cpw_sb = cpw_pool.tile([P, H, D], MMD)

        def outproj_tile(i):
            # rows 128*i..128*(i+1) of this core's shard = chunks 2i, 2i+1
            o_sb = osb_pool.tile([P, D], F32, tag="osb", name=f"osb{i}")
            for dh in range(D // TCH):
                ps_o = ps.tile([P, TCH], F32, tag="ps")
                for h in range(H):
                    nc.tensor.matmul(
                        ps_o[:], yall_p[i][:, h, :, :],
                        cpw_sb[:, h, dh * TCH:(dh + 1) * TCH],
                        start=(h == 0), stop=(h == H - 1))
                osl = o_sb[:, dh * TCH:(dh + 1) * TCH]
                if dh:
                    nc.vector.tensor_copy(osl, ps_o[:])
                else:
                    nc.scalar.copy(osl, ps_o[:])
                nc.sync.dma_start(
                    out=y_shard.ap()[i * P:(i + 1) * P,
                                     dh * TCH:(dh + 1) * TCH],
                    in_=osl)

        # ---- main loop: attn(g-1) with norm(g) + qkv(g+2) + outproj units
        # interleaved between its blocks ----
        state = {}
        for gg in range(3):
            state[gg] = qkv_pre(gg)
            for ii in range(4):
                qkv_tile(gg, state[gg], ii)
        norm_pre(0, state[0])
        for ii in range(4):
            norm_tile(0, state[0], ii)

        for g in range(1, NC_CH):
            feed = [lambda g=g: norm_pre(g, state[g])]
            feed += [lambda g=g, ii=ii: norm_tile(g, state[g], ii)
                     for ii in range(4)]
            if g + 2 < NC_CH:
                state[g + 2] = qkv_pre(g + 2)
                feed += [lambda g=g, ii=ii: qkv_tile(g + 2, state[g + 2], ii)
                         for ii in range(4)]
            if g == 2:  # prefetch output-projection weights mid-flight
                nc.gpsimd.dma_start(
                    out=cpw_sb[:],
                    in_=cpw.ap().rearrange("(h p) d -> p h d", p=P))
            if g >= 6:  # chunks 2i,2i+1 resharded long ago by now
                feed.append(lambda g=g: outproj_tile(g - 6))
            attn_chunk(g - 1, feed)
        attn_chunk(NC_CH - 1)
        flush_den()
        # tile 2's matmuls cover the last chunk's AllToAll latency; tile 3
        # (chunks 6,7) then finds its data already landed.
        outproj_tile(2)
        outproj_tile(3)

    nc.compile()
    return nc


def _host_prep(x, ve, qkv_w, lambdas, c_proj_w):
    x = np.asarray(x, dtype=np.float32)
    ve = np.asarray(ve, dtype=np.float32)
    qkv_w = np.asarray(qkv_w, dtype=np.float32)
    lambdas = np.asarray(lambdas, dtype=np.float32)
    c_proj_w = np.asarray(c_proj_w, dtype=np.float32)

    xT = np.ascontiguousarray(x[0].T.astype(NP_MMD))
    cpwT = np.ascontiguousarray(c_proj_w.T.astype(NP_MMD))
    lam_b = np.ascontiguousarray(np.broadcast_to(lambdas, (P, 2)))

    angular = (np.float32(1.0 / 1024.0)
               ** np.linspace(0.0, 1.0, QUARTER, dtype=np.float32))
    t = np.arange(T, dtype=np.float32)
    theta = t[:, None] * angular[None, :]
    # [T, Q] -> [P, NT*Q] with (p, n*Q+e) = table[n*P+p, e]
    cos32 = np.ascontiguousarray(
        np.cos(theta).astype(NP_MMD).reshape(NT, P, QUARTER)
        .transpose(1, 0, 2).reshape(P, NT * QUARTER))
    sin32 = np.ascontiguousarray(
        np.sin(theta).astype(NP_MMD).reshape(NT, P, QUARTER)
        .transpose(1, 0, 2).reshape(P, NT * QUARTER))

    in_maps = []
    for h in range(N_CORES):
        sl = slice(h * HD, (h + 1) * HD)
        w_qkvT = np.ascontiguousarray(np.concatenate(
            [qkv_w[0, sl, :].T, qkv_w[1, sl, :].T, qkv_w[2, sl, :].T],
            axis=1).astype(NP_MMD))
        in_maps.append({
            "x_t": xT,
            "w_qkv": w_qkvT,
            "cos_t": cos32,
            "sin_t": sin32,
            "ve_h": np.ascontiguousarray(ve[0][:, sl] * lambdas[1]),
            "lam": lam_b,
            "cpw": cpwT,
        })
    return in_maps


def kernel(x, ve, qkv_w, lambdas, c_proj_w, _trace=False, _trace_kwargs=None):
    if "nc" not in _cached:
        _cached["nc"] = build_module()
    nc = _cached["nc"]
    in_maps = _host_prep(x, ve, qkv_w, lambdas, c_proj_w)
    kw = {}
    if _trace:
        kw = dict(trace=True, **(_trace_kwargs or {}))
    res = run_bass_kernel_spmd(nc, in_maps, core_ids=list(range(N_CORES)),
                               **kw)
    _cached["last_result"] = res
    # core j's shard rows are 64-row slices: shard row 64*c+k holds global
    # row c*512 + j*64 + k (interleaved ownership, see build_module)
    parts = np.stack([res.results[c]["y_shard"] for c in range(N_CORES)])
    out = (parts.reshape(N_CORES, NC_CH, TCH // N_CORES, D)
           .transpose(1, 0, 2, 3).reshape(T, D))
    return out[None].astype(np.float32)
